# revision 22
# baseline (speedup 1.0000x reference)
"""Trainium2 Bass kernel for nn_Decoder_3298534884262.

Decoder step: dual additive attention over two [B,L,H] contexts, a merge
gate, one GRU step, then a [V,H] output projection with log_softmax.

Sharding (8 NeuronCores):
  - batch-parallel (B=64 -> 8 rows/core) for attention/merge/GRU
  - vocab-parallel (V padded to 50264 -> 6283 cols/core) for the output
    projection + log_softmax; h_new is AllGathered on-device, and the
    per-core (max, sumexp) log-softmax partials are AllGathered to form
    the global normalizer on every core.

kernel(**inputs) takes the FULL unsharded inputs (numpy, keyed as in
reference.setup_inputs()) and returns the FULL output tuple
(log_probs [B,V] f32, h_new [1,B,H] f32).
"""

import sys

if "/opt/trn_rl_repo" not in sys.path:
    sys.path.insert(0, "/opt/trn_rl_repo")

import numpy as np

import concourse.bacc as bacc
import concourse.mybir as mybir
import concourse.tile as tile
from concourse.bass_utils import run_bass_kernel_spmd

F32 = mybir.dt.float32
F32R = mybir.dt.float32r
BF16 = mybir.dt.bfloat16
U8 = mybir.dt.uint8

NC = 8          # cores
B = 64
BS = B // NC    # batch rows per core
L = 128
H = 1024
HC = H // 128   # h chunks
E = 512
EC = E // 128
V = 50257
VS = 6284       # vocab cols per core (padded so the tail N-block is even)
VP = VS * NC    # padded vocab
G2 = 2048       # r,z gates width
XD = E + H + H  # 2560: [input; c_t; h] contraction for r,z
KC_X = XD // 128   # 20
KC_IN = (E + H) // 128  # 12 (x part for i_n)
NEG_BIG = -1.0e30


def _f32r(ap):
    return ap.bitcast(F32R)


def _vblocks():
    """Output-projection N-blocks over the 6283 vocab shard."""
    out = []
    o = 0
    while o < VS:
        n = min(512, VS - o)
        out.append((o, n))
        o += n
    return out


def build_kernel():
    nc = bacc.Bacc("TRN2", target_bir_lowering=False, debug=False, num_devices=NC)

    din = {}
    def inp(name, shape, dtype=F32):
        din[name] = nc.dram_tensor(name, list(shape), dtype, kind="ExternalInput")
        return din[name]

    ctxT = inp("ctxT", [2, BS, H, L])
    ctxN = inp("ctxN", [2, BS, L, H])
    hT = inp("hT", [H, BS])
    hN = inp("hN", [BS, H])
    inT = inp("inT", [E, BS])
    pad = inp("pad", [1, BS * L], U8)
    U_d = inp("U", [H, H])
    W_d = inp("W", [H, H])
    v_d = inp("v", [H, 1])
    wsh_t = inp("wsh_t", [H, H])
    wsc_t = inp("wsc_t", [H, H])
    wsr_t = inp("wsr_t", [H, H])
    bsh_d = inp("bsh", [H])
    bsc_d = inp("bsc", [H])
    bsr_d = inp("bsr", [H])
    wS_d = inp("wS", [H, 1])
    w_rz = inp("w_rz_t", [XD, G2])
    w_in = inp("w_in_t", [E + H, H])
    w_hn = inp("w_hn_t", [H, H])
    b_rz = inp("b_rz", [2, G2])
    b_in = inp("b_in", [1, H])
    b_hn = inp("b_hn", [1, H])
    wout = inp("w_out_t", [H, VS])
    bout = inp("b_out", [1, VS])

    logp_d = nc.dram_tensor("logp", [B, VS], F32, kind="ExternalOutput")
    hnew_d = nc.dram_tensor("hnew", [BS, H], F32, kind="ExternalOutput")

    rg = [list(range(NC))]

    with tile.TileContext(nc) as tc:
        with (
            tc.tile_pool(name="const", bufs=1) as pc,
            tc.tile_pool(name="persist", bufs=1) as pp,
            tc.tile_pool(name="dram", bufs=1, space="DRAM") as pdram,
        ):
            # ---- constants / persistent small tiles ----
            ones = pc.tile([2, 128], F32, tag="ones")
            nc.gpsimd.memset(ones[:], 1.0)
            negbig = pc.tile([1, L], F32, tag="negbig")
            nc.gpsimd.memset(negbig[:], -100000.0)

            hT_sb = pp.tile([128, HC, BS], F32, tag="hT")
            nc.sync.dma_start(hT_sb[:], hT.ap().rearrange("(c p) i -> p c i", p=128))
            hT_r = pp.tile([128, HC, BS], F32R, tag="hTr")
            nc.sync.dma_start(hT_r[:], hT.ap().bitcast(F32R).rearrange("(c p) i -> p c i", p=128))
            inT_sb = pp.tile([128, EC, BS], F32, tag="inT")
            nc.sync.dma_start(inT_sb[:], inT.ap().rearrange("(c p) i -> p c i", p=128))
            pad_sb = pp.tile([1, BS * L], U8, tag="pad")
            nc.sync.dma_start(pad_sb[:], pad.ap())
            v_sb = pp.tile([128, HC], F32, tag="v")
            nc.sync.dma_start(v_sb[:], v_d.ap().rearrange("(c p) o -> p (c o)", p=128))
            v_bf = pp.tile([128, HC], BF16, tag="vbf")
            nc.vector.tensor_copy(v_bf[:], v_sb[:])
            wS_sb = pp.tile([128, HC], F32, tag="wS")
            nc.sync.dma_start(wS_sb[:], wS_d.ap().rearrange("(c p) o -> p (c o)", p=128))
            bsh_sb = pp.tile([128, HC], F32, tag="bsh")
            nc.sync.dma_start(bsh_sb[:], bsh_d.ap().rearrange("(c p) -> p c", p=128))
            bsc_sb = pp.tile([128, HC], F32, tag="bsc")
            nc.sync.dma_start(bsc_sb[:], bsc_d.ap().rearrange("(c p) -> p c", p=128))
            bsr_sb = pp.tile([128, HC], F32, tag="bsr")
            nc.sync.dma_start(bsr_sb[:], bsr_d.ap().rearrange("(c p) -> p c", p=128))
            bsum_c = pp.tile([128, HC], F32, tag="bsum_c")
            nc.vector.tensor_tensor(bsum_c[:], bsh_sb[:], bsc_sb[:], mybir.AluOpType.add)
            bsum_r = pp.tile([128, HC], F32, tag="bsum_r")
            nc.vector.tensor_tensor(bsum_r[:], bsh_sb[:], bsr_sb[:], mybir.AluOpType.add)

            U_sb = pp.tile([128, HC, H], F32R, tag="U")
            nc.sync.dma_start(U_sb[:], U_d.ap().bitcast(F32R).rearrange("(c p) n -> p c n", p=128))

            # attention outputs, transposed: [128, hc, (j, i)]
            attT = pp.tile([128, HC, 2, BS], F32, tag="attT")
            # GRU transposed input stack (f32r): slots 0..3 = input, 4..11 = c_t
            zinT = pp.tile([128, KC_IN, BS], F32R, tag="zinT")
            nc.vector.tensor_copy(zinT[:, 0:EC, :], inT_sb[:])

            def zin(kc):
                return zinT[:, kc, :] if kc < KC_IN else hT_r[:, kc - KC_IN, :]

            # =======================================================
            # Phase A0: ahT[h, i] = (W.T @ hidden_shard.T)  [H, BS]
            # =======================================================
            ahT_sb = pp.tile([128, HC * BS], F32, tag="ahT")
            with (
                tc.tile_pool(name="wstream", bufs=2) as pw,
                tc.tile_pool(name="psA0", bufs=1, space="PSUM") as psA0,
            ):
                ps_ah = psA0.tile([128, HC * BS], F32, tag="pah")
                for kc in range(HC):
                    wt = pw.tile([128, H], F32, tag="wtile")
                    nc.sync.dma_start(wt[:], W_d.ap()[kc * 128:(kc + 1) * 128, :])
                    for hc in range(HC):
                        nc.tensor.matmul(
                            ps_ah[:, hc * BS:(hc + 1) * BS],
                            lhsT=wt[:, hc * 128:(hc + 1) * 128],
                            rhs=hT_sb[:, kc, :],
                            start=(kc == 0), stop=(kc == HC - 1),
                        )
                nc.scalar.copy(ahT_sb[:], ps_ah[:])

            # =======================================================
            # Phase A: attention for j in {cnn, rnn}, batch pairs
            # =======================================================
            with (
                tc.tile_pool(name="actx", bufs=10) as pctx,
                tc.tile_pool(name="actxN", bufs=3) as pctxN,
                tc.tile_pool(name="atanh", bufs=3) as ptanh,
                tc.tile_pool(name="asmall", bufs=8) as psm,
                tc.tile_pool(name="psCU", bufs=2, space="PSUM") as psCU,
                tc.tile_pool(name="psSC", bufs=2, space="PSUM") as psSC,
                tc.tile_pool(name="psAT", bufs=1, space="PSUM") as psAT,
            ):
                for j in range(2):
                    for p in range(BS // 2):
                        i0 = 2 * p
                        # transposed context tiles for the pair, per k-chunk
                        ctx_t = []
                        for kc in range(HC):
                            t = pctx.tile([128, 2, L], F32R, tag=f"ctxT{kc % 2}")
                            nc.sync.dma_start(
                                t[:],
                                ctxT.ap().bitcast(F32R)[j, i0:i0 + 2, kc * 128:(kc + 1) * 128, :]
                                .rearrange("i p l -> p i l"),
                            )
                            ctx_t.append(t)
                        # scores psum for the pair: [1, 2*L]
                        ps_sc = psSC.tile([1, 2 * L], F32, tag="score")
                        for hc in range(HC):
                            ps_cu = psCU.tile([128, 2 * L], F32, tag="cu")
                            for kc in range(HC):
                                nc.tensor.matmul(
                                    ps_cu[:],
                                    lhsT=U_sb[:, kc, hc * 128:(hc + 1) * 128],
                                    rhs=ctx_t[kc][:],
                                    start=(kc == 0), stop=(kc == HC - 1),
                                )
                            th = ptanh.tile([128, 2 * L], BF16, tag="tanh")
                            for ii in range(2):
                                nc.scalar.activation(
                                    th[:, ii * L:(ii + 1) * L],
                                    ps_cu[:, ii * L:(ii + 1) * L],
                                    mybir.ActivationFunctionType.Tanh,
                                    bias=ahT_sb[:, hc * BS + i0 + ii:hc * BS + i0 + ii + 1],
                                )
                            nc.tensor.matmul(
                                ps_sc[:],
                                lhsT=v_bf[:, hc:hc + 1],
                                rhs=th[:],
                                start=(hc == 0), stop=(hc == HC - 1),
                            )
                        for ii in range(2):
                            i = i0 + ii
                            # softmax over L (unnormalized weights + recip sum)
                            srow = psm.tile([1, L], F32, tag="srow")
                            nc.scalar.copy(srow[:], ps_sc[:, ii * L:(ii + 1) * L])
                            nc.vector.copy_predicated(srow[:], pad_sb[0:1, i * L:(i + 1) * L],
                                                      negbig[:])
                            nmax = psm.tile([1, 1], F32, tag="nmax")
                            nc.vector.tensor_reduce(
                                nmax[:], srow[:], axis=mybir.AxisListType.X,
                                op=mybir.AluOpType.max, negate=True,
                            )
                            wrow = psm.tile([1, L], F32, tag="wrow")
                            ssum = psm.tile([1, 1], F32, tag="ssum")
                            nc.scalar.activation(
                                wrow[:], srow[:], mybir.ActivationFunctionType.Exp,
                                bias=nmax[:], accum_out=ssum[:],
                            )
                            rsum = psm.tile([1, 1], F32, tag="rsum")
                            nc.vector.reciprocal(rsum[:], ssum[:])
                            # wcol = wrow.T * (1/sum)  via PE: lhsT=wrow, rhs=rsum
                            ps_wc = psAT.tile([128, 1], F32, tag="wcol_ps")
                            nc.tensor.matmul(ps_wc[:], lhsT=wrow[:], rhs=rsum[:],
                                             start=True, stop=True)
                            wcol = psm.tile([128, 1], F32, tag="wcol")
                            nc.vector.tensor_copy(wcol[:], ps_wc[:])
                            # natural ctx for this (j, i)
                            cn = pctxN.tile([L, H], F32, tag="ctxN")
                            nc.sync.dma_start(cn[:], ctxN.ap()[j, i, :, :])
                            # attT[:, hc, j, i] = ctxN_chunk.T @ wcol
                            ps_at = psAT.tile([128, HC], F32, tag="attT_ps", bufs=2)
                            for hc in range(HC):
                                nc.tensor.matmul(
                                    ps_at[:, hc:hc + 1],
                                    lhsT=cn[:, hc * 128:(hc + 1) * 128],
                                    rhs=wcol[:],
                                    start=True, stop=True,
                                )
                            for hc in range(HC):
                                nc.vector.tensor_copy(attT[:, hc, j, i:i + 1], ps_at[:, hc:hc + 1])

            # =======================================================
            # Phase B: merge gate (T-space)
            # =======================================================
            with (
                tc.tile_pool(name="mw", bufs=3) as pmw,
                tc.tile_pool(name="msb", bufs=2) as pmsb,
                tc.tile_pool(name="psM", bufs=1, space="PSUM") as psM,
                tc.tile_pool(name="psS2", bufs=1, space="PSUM") as psS2,
            ):
                ps_sh = psM.tile([128, HC, BS], F32, tag="sh")
                ps_c = psM.tile([128, HC, BS], F32, tag="sc")
                ps_r = psM.tile([128, HC, BS], F32, tag="sr")
                for kc in range(HC):
                    wh = pmw.tile([128, H], F32, tag="wh")
                    nc.sync.dma_start(wh[:], wsh_t.ap()[kc * 128:(kc + 1) * 128, :])
                    wc = pmw.tile([128, H], F32, tag="wc")
                    nc.sync.dma_start(wc[:], wsc_t.ap()[kc * 128:(kc + 1) * 128, :])
                    wr = pmw.tile([128, H], F32, tag="wr")
                    nc.sync.dma_start(wr[:], wsr_t.ap()[kc * 128:(kc + 1) * 128, :])
                    st, sp = (kc == 0), (kc == HC - 1)
                    for hc in range(HC):
                        nc.tensor.matmul(ps_sh[:, hc, :], lhsT=wh[:, hc * 128:(hc + 1) * 128],
                                         rhs=hT_sb[:, kc, :], start=st, stop=sp)
                        nc.tensor.matmul(ps_c[:, hc, :], lhsT=wc[:, hc * 128:(hc + 1) * 128],
                                         rhs=attT[:, kc, 0, :], start=st, stop=sp)
                        nc.tensor.matmul(ps_r[:, hc, :], lhsT=wr[:, hc * 128:(hc + 1) * 128],
                                         rhs=attT[:, kc, 1, :], start=st, stop=sp)
                # tanh(c-branch), tanh(r-branch); wS dot partial scores
                sh_sb = pmsb.tile([128, HC, BS], F32, tag="sh_sb")
                nc.scalar.copy(sh_sb[:], ps_sh[:])
                tnh = pmsb.tile([128, HC, 2 * BS], F32, tag="tnh")
                for hc in range(HC):
                    for (br, ps_b, bias) in ((0, ps_c, bsum_c), (1, ps_r, bsum_r)):
                        tmp = tnh[:, hc, br * BS:(br + 1) * BS]
                        nc.vector.tensor_tensor(tmp, ps_b[:, hc, :], sh_sb[:, hc, :],
                                                mybir.AluOpType.add)
                        nc.scalar.activation(tmp, tmp, mybir.ActivationFunctionType.Tanh,
                                             bias=bias[:, hc:hc + 1])
                ps_s2 = psS2.tile([1, 2 * BS], F32, tag="s2")
                for hc in range(HC):
                    nc.tensor.matmul(ps_s2[:], lhsT=wS_sb[:, hc:hc + 1],
                                     rhs=tnh[:, hc, :], start=(hc == 0), stop=(hc == HC - 1))
                s2_sb = pmsb.tile([1, 2 * BS], F32, tag="s2_sb")
                nc.scalar.copy(s2_sb[:], ps_s2[:])
                gdiff = pmsb.tile([1, BS], F32, tag="gdiff")
                nc.vector.tensor_tensor(gdiff[:], s2_sb[:, 0:BS], s2_sb[:, BS:2 * BS],
                                        mybir.AluOpType.subtract)
                grow = pmsb.tile([1, BS], F32, tag="grow")
                nc.scalar.activation(grow[:], gdiff[:], mybir.ActivationFunctionType.Sigmoid)
                ps_gb = psS2.tile([128, BS], F32, tag="gb")
                nc.tensor.matmul(ps_gb[:], lhsT=ones[0:1, 0:128], rhs=grow[:],
                                 start=True, stop=True)
                gb = pmsb.tile([128, BS], F32, tag="gb_sb")
                nc.vector.tensor_copy(gb[:], ps_gb[:])
                # c_tT = arT + gb*(acT - arT) -> into zinT slots 4..11 (f32r)
                for hc in range(HC):
                    ctmp = pmsb.tile([128, BS], F32, tag="ctmp", bufs=3)
                    nc.vector.tensor_tensor(ctmp[:], attT[:, hc, 0, :], attT[:, hc, 1, :],
                                            mybir.AluOpType.subtract)
                    nc.vector.tensor_tensor(ctmp[:], ctmp[:], gb[:], mybir.AluOpType.mult)
                    nc.vector.tensor_tensor(zinT[:, EC + hc, :], ctmp[:], attT[:, hc, 1, :],
                                            mybir.AluOpType.add)

            # =======================================================
            # Phase C: GRU step (natural space, 8 rows)
            # =======================================================
            hnew_sb = pp.tile([BS, H], F32, tag="hnew")
            with (
                tc.tile_pool(name="gw", bufs=4) as pgw,
                tc.tile_pool(name="gsb", bufs=1) as pgsb,
                tc.tile_pool(name="psG", bufs=3, space="PSUM") as psG,
            ):
                brz_sb = pgsb.tile([2, G2], F32, tag="brz")
                nc.sync.dma_start(brz_sb[:], b_rz.ap())
                bin_sb = pgsb.tile([1, H], F32, tag="bin")
                nc.sync.dma_start(bin_sb[:], b_in.ap())
                bhn_sb = pgsb.tile([1, H], F32, tag="bhn")
                nc.sync.dma_start(bhn_sb[:], b_hn.ap())
                hN_sb = pgsb.tile([BS, H], F32, tag="hN")
                nc.sync.dma_start(hN_sb[:], hN.ap())

                rz_sb = pgsb.tile([BS, G2], F32, tag="rz")
                for nb in range(4):
                    ps = psG.tile([BS, 512], F32, tag="gps")
                    for kc in range(KC_X):
                        wt = pgw.tile([128, 512], F32R, tag="gw")
                        nc.sync.dma_start(
                            wt[:], w_rz.ap().bitcast(F32R)[kc * 128:(kc + 1) * 128,
                                                           nb * 512:(nb + 1) * 512])
                        nc.tensor.matmul(ps[:], lhsT=zin(kc), rhs=wt[:],
                                         start=(kc == 0), stop=False)
                    nc.tensor.matmul(ps[:], lhsT=ones[0:2, 0:BS],
                                     rhs=brz_sb[:, nb * 512:(nb + 1) * 512],
                                     start=False, stop=True)
                    nc.scalar.activation(rz_sb[:, nb * 512:(nb + 1) * 512], ps[:],
                                         mybir.ActivationFunctionType.Sigmoid)
                n_sb = pgsb.tile([BS, H], F32, tag="n_sb")
                for nb in range(2):
                    ps_i = psG.tile([BS, 512], F32, tag="gps")
                    for kc in range(KC_IN):
                        wt = pgw.tile([128, 512], F32R, tag="gw")
                        nc.sync.dma_start(
                            wt[:], w_in.ap().bitcast(F32R)[kc * 128:(kc + 1) * 128,
                                                           nb * 512:(nb + 1) * 512])
                        nc.tensor.matmul(ps_i[:], lhsT=zin(kc), rhs=wt[:],
                                         start=(kc == 0), stop=False)
                    nc.tensor.matmul(ps_i[:], lhsT=ones[0:1, 0:BS],
                                     rhs=bin_sb[:, nb * 512:(nb + 1) * 512],
                                     start=False, stop=True)
                    ps_h = psG.tile([BS, 512], F32, tag="gps")
                    for kc in range(HC):
                        wt = pgw.tile([128, 512], F32R, tag="gw")
                        nc.sync.dma_start(
                            wt[:], w_hn.ap().bitcast(F32R)[kc * 128:(kc + 1) * 128,
                                                           nb * 512:(nb + 1) * 512])
                        nc.tensor.matmul(ps_h[:], lhsT=hT_r[:, kc, :], rhs=wt[:],
                                         start=(kc == 0), stop=False)
                    nc.tensor.matmul(ps_h[:], lhsT=ones[0:1, 0:BS],
                                     rhs=bhn_sb[:, nb * 512:(nb + 1) * 512],
                                     start=False, stop=True)
                    sl = slice(nb * 512, (nb + 1) * 512)
                    # n = tanh(i_n + r * h_n)
                    nc.vector.tensor_tensor(n_sb[:, sl], ps_h[:], rz_sb[:, sl],
                                            mybir.AluOpType.mult)
                    nc.vector.tensor_tensor(n_sb[:, sl], n_sb[:, sl], ps_i[:],
                                            mybir.AluOpType.add)
                    nc.scalar.activation(n_sb[:, sl], n_sb[:, sl],
                                         mybir.ActivationFunctionType.Tanh)
                # h_new = n + z*(h - n)
                tdif = pgsb.tile([BS, H], F32, tag="tdif")
                nc.vector.tensor_tensor(tdif[:], hN_sb[:], n_sb[:], mybir.AluOpType.subtract)
                nc.vector.tensor_tensor(tdif[:], tdif[:], rz_sb[:, H:2 * H],
                                        mybir.AluOpType.mult)
                nc.vector.tensor_tensor(hnew_sb[:], n_sb[:], tdif[:], mybir.AluOpType.add)
                nc.sync.dma_start(hnew_d.ap(), hnew_sb[:])

            # =======================================================
            # Phase D: AllGather h_new; output projection + log_softmax
            # =======================================================
            ag_in = pdram.tile([BS, H], F32, tag="ag_in")
            ag_out = pdram.tile([B, H], F32, tag="ag_out")
            nc.sync.dma_start(ag_in[:], hnew_sb[:])
            nc.gpsimd.collective_compute(
                "AllGather", mybir.AluOpType.bypass, replica_groups=rg,
                ins=[ag_in.opt()], outs=[ag_out.opt()],
            )
            with (
                tc.tile_pool(name="dsb", bufs=1) as pdsb,
                tc.tile_pool(name="wout", bufs=6) as pwo,
                tc.tile_pool(name="dscr", bufs=3) as pscr,
                tc.tile_pool(name="psT", bufs=2, space="PSUM") as psT,
                tc.tile_pool(name="psL", bufs=3, space="PSUM") as psL,
            ):
                hfull = pdsb.tile([B, H], F32, tag="hfull")
                nc.sync.dma_start(hfull[:], ag_out[:])
                # transpose h_new: hT64[:, kc, :] = hfull[:, kc*128:+128].T
                idn = pdsb.tile([B, B], F32, tag="idn")
                from concourse import masks
                masks.make_identity(nc, idn[:])
                hT64 = pdsb.tile([128, HC, B], F32R, tag="hT64")
                for kc in range(HC):
                    ps_t = psT.tile([128, B], F32, tag="pst")
                    nc.tensor.matmul(ps_t[:], lhsT=hfull[:, kc * 128:(kc + 1) * 128],
                                     rhs=idn[:], is_transpose=True, start=True, stop=True)
                    nc.vector.tensor_copy(hT64[:, kc, :], ps_t[:])

                bout_sb = pdsb.tile([1, VS], F32, tag="bout")
                nc.sync.dma_start(bout_sb[:], bout.ap())
                logits = pdsb.tile([B, VS], F32, tag="logits")
                vb = _vblocks()
                nbv = len(vb)
                mx = pdsb.tile([B, nbv], F32, tag="mx")
                sx = pdsb.tile([B, nbv], F32, tag="sx")
                for bi, (o, n) in enumerate(vb):
                    ps_l = psL.tile([B, 512], F32, tag="lps")
                    for kc in range(HC):
                        wt = pwo.tile([128, 512], F32R, tag="wot")
                        nc.sync.dma_start(wt[:, 0:n],
                                          wout.ap().bitcast(F32R)[kc * 128:(kc + 1) * 128, o:o + n])
                        nc.tensor.matmul(ps_l[:, 0:n], lhsT=hT64[:, kc, :],
                                         rhs=wt[:, 0:n], start=(kc == 0), stop=False)
                    nc.tensor.matmul(ps_l[:, 0:n], lhsT=ones[0:1, 0:B],
                                     rhs=bout_sb[:, o:o + n], start=False, stop=True)
                    nc.scalar.copy(logits[:, o:o + n], ps_l[:, 0:n])
                    nc.vector.tensor_reduce(mx[:, bi:bi + 1], ps_l[:, 0:n],
                                            axis=mybir.AxisListType.X, op=mybir.AluOpType.max)
                nmx = pdsb.tile([B, 1], F32, tag="nmx")
                nc.vector.tensor_reduce(nmx[:], mx[:], axis=mybir.AxisListType.X,
                                        op=mybir.AluOpType.max, negate=True)
                pmx = pdsb.tile([B, 1], F32, tag="pmx")
                nc.scalar.mul(pmx[:], nmx[:], -1.0)
                for bi, (o, n) in enumerate(vb):
                    scr = pscr.tile([B, 512], F32, tag="scr")
                    nc.scalar.activation(scr[:, 0:n], logits[:, o:o + n],
                                         mybir.ActivationFunctionType.Exp,
                                         bias=nmx[:], accum_out=sx[:, bi:bi + 1])
                sloc = pdsb.tile([B, 1], F32, tag="sloc")
                nc.vector.tensor_reduce(sloc[:], sx[:], axis=mybir.AxisListType.X,
                                        op=mybir.AluOpType.add)
                # pack partials [B, 2] = (max, sumexp); AllGather; combine
                part = pdsb.tile([B, 2], F32, tag="part")
                nc.vector.tensor_copy(part[:, 0:1], pmx[:])
                nc.vector.tensor_copy(part[:, 1:2], sloc[:])
                ag2_in = pdram.tile([B, 2], F32, tag="ag2_in")
                ag2_out = pdram.tile([NC, B, 2], F32, tag="ag2_out")
                nc.sync.dma_start(ag2_in[:], part[:])
                nc.gpsimd.collective_compute(
                    "AllGather", mybir.AluOpType.bypass, replica_groups=rg,
                    ins=[ag2_in.opt()], outs=[ag2_out.opt()],
                )
                gath = pdsb.tile([B, 2, NC], F32, tag="gath")
                nc.sync.dma_start(gath[:], ag2_out[:].rearrange("r b c -> b c r"))
                gnm = pdsb.tile([B, 1], F32, tag="gnm")
                nc.vector.tensor_reduce(gnm[:], gath[:, 0:1, :], axis=mybir.AxisListType.X,
                                        op=mybir.AluOpType.max, negate=True)
                gpm = pdsb.tile([B, 1], F32, tag="gpm")
                nc.scalar.mul(gpm[:], gnm[:], -1.0)
                # sum_r s_r * exp(m_r - M)
                et = pdsb.tile([B, NC], F32, tag="et")
                nc.scalar.activation(et[:], gath[:, 0, :], mybir.ActivationFunctionType.Exp,
                                     bias=gnm[:])
                nc.vector.tensor_tensor(et[:], et[:], gath[:, 1, :], mybir.AluOpType.mult)
                gs = pdsb.tile([B, 1], F32, tag="gs")
                nc.vector.tensor_reduce(gs[:], et[:], axis=mybir.AxisListType.X,
                                        op=mybir.AluOpType.add)
                lng = pdsb.tile([B, 1], F32, tag="lng")
                nc.scalar.activation(lng[:], gs[:], mybir.ActivationFunctionType.Ln)
                nlz = pdsb.tile([B, 1], F32, tag="nlz")
                nc.vector.tensor_tensor(nlz[:], gpm[:], lng[:], mybir.AluOpType.add)
                nc.scalar.mul(nlz[:], nlz[:], -1.0)
                # logp = logits - logZ ; single pass then DMA out
                nc.scalar.activation(logits[:], logits[:],
                                     mybir.ActivationFunctionType.Identity, bias=nlz[:])
                nc.sync.dma_start(logp_d.ap(), logits[:])

    nc.finalize()
    return nc


_NC_CACHE = None


def _get_nc():
    global _NC_CACHE
    if _NC_CACHE is None:
        _NC_CACHE = build_kernel()
    return _NC_CACHE


def make_in_maps(inputs):
    """Shard + lay out the full inputs into per-core input maps."""
    f = np.ascontiguousarray
    inp = np.asarray(inputs["input"], np.float32)          # [B,1,E]
    hid = np.asarray(inputs["hidden"], np.float32)         # [1,B,H]
    cc = np.asarray(inputs["context_hiddens_cnn"], np.float32)
    cr = np.asarray(inputs["context_hiddens_rnn"], np.float32)
    pad = np.asarray(inputs["pad_matrix"]).astype(np.uint8)
    W = np.asarray(inputs["W"], np.float32)
    U = np.asarray(inputs["U"], np.float32)
    v = np.asarray(inputs["v"], np.float32)
    WSh_w = np.asarray(inputs["WSh_w"], np.float32)
    WSh_b = np.asarray(inputs["WSh_b"], np.float32)
    WSc_w = np.asarray(inputs["WSc_w"], np.float32)
    WSc_b = np.asarray(inputs["WSc_b"], np.float32)
    WSr_w = np.asarray(inputs["WSr_w"], np.float32)
    WSr_b = np.asarray(inputs["WSr_b"], np.float32)
    wS_w = np.asarray(inputs["wS_w"], np.float32)
    W_ih = np.asarray(inputs["W_ih"], np.float32)
    W_hh = np.asarray(inputs["W_hh"], np.float32)
    b_ih = np.asarray(inputs["b_ih"], np.float32)
    b_hh = np.asarray(inputs["b_hh"], np.float32)
    W_out = np.asarray(inputs["W_out"], np.float32)
    b_out = np.asarray(inputs["b_out"], np.float32)

    # shared (replicated) weight layouts
    wsh_t = f(WSh_w.T)
    wsc_t = f(WSc_w.T)
    wsr_t = f(WSr_w.T)
    wS_col = f(wS_w[0][:, None])
    w_rz_t = f(np.concatenate([W_ih[:G2, :], W_hh[:G2, :]], axis=1).T)  # [2560, 2048]
    w_in_t = f(W_ih[G2:, :].T)    # [1536, 1024]
    w_hn_t = f(W_hh[G2:, :].T)    # [1024, 1024]
    b_rz2 = f(np.stack([b_ih[:G2], b_hh[:G2]], axis=0))
    b_in1 = f(b_ih[None, G2:])
    b_hn1 = f(b_hh[None, G2:])

    # padded vocab shards
    Wout_p = np.zeros((VP, H), np.float32)
    Wout_p[:V] = W_out
    bout_p = np.full((VP,), NEG_BIG, np.float32)
    bout_p[:V] = b_out

    ctx2 = np.stack([cc, cr], axis=0)  # [2, B, L, H]

    maps = []
    for k in range(NC):
        bs = slice(k * BS, (k + 1) * BS)
        vs = slice(k * VS, (k + 1) * VS)
        m = {
            "ctxT": f(ctx2[:, bs].transpose(0, 1, 3, 2)),
            "ctxN": f(ctx2[:, bs]),
            "hT": f(hid[0, bs].T),
            "hN": f(hid[0, bs]),
            "inT": f(inp[bs, 0, :].T),
            "pad": f(pad[bs].reshape(1, -1)),
            "U": U, "W": W, "v": v,
            "wsh_t": wsh_t, "wsc_t": wsc_t, "wsr_t": wsr_t,
            "bsh": WSh_b, "bsc": WSc_b, "bsr": WSr_b,
            "wS": wS_col,
            "w_rz_t": w_rz_t, "w_in_t": w_in_t, "w_hn_t": w_hn_t,
            "b_rz": b_rz2, "b_in": b_in1, "b_hn": b_hn1,
            "w_out_t": f(Wout_p[vs].T),
            "b_out": f(bout_p[None, vs]),
        }
        maps.append(m)
    return maps


def assemble(results):
    logp = np.empty((B, VP), np.float32)
    hnew = np.empty((B, H), np.float32)
    for k in range(NC):
        logp[:, k * VS:(k + 1) * VS] = results[k]["logp"]
        hnew[k * BS:(k + 1) * BS] = results[k]["hnew"]
    return logp[:, :V], hnew[None]


def kernel(**inputs):
    nc = _get_nc()
    in_maps = make_in_maps(inputs)
    res = run_bass_kernel_spmd(nc, in_maps, core_ids=list(range(NC)))
    return assemble(res.results)


# revision 34
# speedup vs baseline: 1.0005x; 1.0005x over previous
"""Trainium2 Bass kernel for nn_Decoder_3298534884262.

Decoder step: dual additive attention over two [B,L,H] contexts, a merge
gate, one GRU step, then a [V,H] output projection with log_softmax.

Sharding (8 NeuronCores):
  - batch-parallel (B=64 -> 8 rows/core) for attention/merge/GRU
  - vocab-parallel (V padded to 50264 -> 6283 cols/core) for the output
    projection + log_softmax; h_new is AllGathered on-device, and the
    per-core (max, sumexp) log-softmax partials are AllGathered to form
    the global normalizer on every core.

kernel(**inputs) takes the FULL unsharded inputs (numpy, keyed as in
reference.setup_inputs()) and returns the FULL output tuple
(log_probs [B,V] f32, h_new [1,B,H] f32).
"""

import sys

if "/opt/trn_rl_repo" not in sys.path:
    sys.path.insert(0, "/opt/trn_rl_repo")

import numpy as np

import concourse.bacc as bacc
import concourse.mybir as mybir
import concourse.tile as tile
from concourse.bass_utils import run_bass_kernel_spmd

F32 = mybir.dt.float32
F32R = mybir.dt.float32r
BF16 = mybir.dt.bfloat16
U8 = mybir.dt.uint8

NC = 8          # cores
B = 64
BS = B // NC    # batch rows per core
L = 128
H = 1024
HC = H // 128   # h chunks
E = 512
EC = E // 128
V = 50257
VS = 6284       # vocab cols per core (padded so the tail N-block is even)
VP = VS * NC    # padded vocab
G2 = 2048       # r,z gates width
XD = E + H + H  # 2560: [input; c_t; h] contraction for r,z
KC_X = XD // 128   # 20
KC_IN = (E + H) // 128  # 12 (x part for i_n)
NEG_BIG = -1.0e30


def _f32r(ap):
    return ap.bitcast(F32R)


def _vblocks():
    """Output-projection N-blocks over the 6283 vocab shard."""
    out = []
    o = 0
    while o < VS:
        n = min(512, VS - o)
        out.append((o, n))
        o += n
    return out


def build_kernel():
    nc = bacc.Bacc("TRN2", target_bir_lowering=False, debug=False, num_devices=NC)

    din = {}
    def inp(name, shape, dtype=F32):
        din[name] = nc.dram_tensor(name, list(shape), dtype, kind="ExternalInput")
        return din[name]

    ctxT = inp("ctxT", [2, BS, H, L])
    ctxN = inp("ctxN", [2, BS, L, H])
    hT = inp("hT", [H, BS])
    hTf = inp("hTf", [H, B])         # full hidden transposed (for TP-GRU)
    inTf = inp("inTf", [E, B])       # full input transposed (for TP-GRU)
    hcol = inp("hcol", [B, 128])     # full hidden, this core's H-column shard
    pad = inp("pad", [1, BS * L], U8)
    U_d = inp("U", [H, H])
    W_d = inp("W", [H, H])
    v_d = inp("v", [H, 1])
    wsh_t = inp("wsh_t", [H, H])
    wsc_t = inp("wsc_t", [H, H])
    wsr_t = inp("wsr_t", [H, H])
    bsh_d = inp("bsh", [H])
    bsc_d = inp("bsc", [H])
    bsr_d = inp("bsr", [H])
    wS_d = inp("wS", [H, 1])
    # TP-GRU weight shards (columns of the transposed weights)
    w_rz = inp("w_rz_k", [XD, 256])      # [r_shard | z_shard]
    w_in = inp("w_in_k", [E + H, 128])
    w_hn = inp("w_hn_k", [H, 128])
    b_rz = inp("b_rz_k", [2, 256])
    b_in = inp("b_in_k", [1, 128])
    b_hn = inp("b_hn_k", [1, 128])
    wout = inp("w_out_t", [H, VS])
    bout = inp("b_out", [1, VS])

    logp_d = nc.dram_tensor("logp", [B, VS], F32, kind="ExternalOutput")
    hnew_d = nc.dram_tensor("hnewc", [B, 128], F32, kind="ExternalOutput")

    rg = [list(range(NC))]

    with tile.TileContext(nc) as tc:
        with (
            tc.tile_pool(name="const", bufs=1) as pc,
            tc.tile_pool(name="persist", bufs=1) as pp,
            tc.tile_pool(name="dram", bufs=1, space="DRAM") as pdram,
        ):
            # ---- constants / persistent small tiles ----
            ones = pc.tile([2, 128], F32, tag="ones")
            nc.gpsimd.memset(ones[:], 1.0)
            ones_r = pc.tile([2, 128], F32R, tag="ones_r")
            nc.vector.tensor_copy(ones_r[:], ones[:])
            negbig = pc.tile([1, L], F32, tag="negbig")
            nc.gpsimd.memset(negbig[:], -100000.0)
            from concourse import masks
            idn = pc.tile([B, B], F32, tag="idn")
            masks.make_identity(nc, idn[:])

            hT_sb = pp.tile([128, HC, BS], F32, tag="hT")
            nc.sync.dma_start(hT_sb[:], hT.ap().rearrange("(c p) i -> p c i", p=128))
            hTf_r = pp.tile([128, HC, B], F32R, tag="hTfr")
            nc.sync.dma_start(hTf_r[:], hTf.ap().bitcast(F32R).rearrange("(c p) i -> p c i", p=128))
            pad_sb = pp.tile([1, BS * L], U8, tag="pad")
            nc.sync.dma_start(pad_sb[:], pad.ap())
            v_sb = pp.tile([128, HC], F32, tag="v")
            nc.sync.dma_start(v_sb[:], v_d.ap().rearrange("(c p) o -> p (c o)", p=128))
            v_bf = pp.tile([128, HC], BF16, tag="vbf")
            nc.vector.tensor_copy(v_bf[:], v_sb[:])
            wS_sb = pp.tile([128, HC], F32, tag="wS")
            nc.sync.dma_start(wS_sb[:], wS_d.ap().rearrange("(c p) o -> p (c o)", p=128))
            bsh_sb = pp.tile([128, HC], F32, tag="bsh")
            nc.sync.dma_start(bsh_sb[:], bsh_d.ap().rearrange("(c p) -> p c", p=128))
            bsc_sb = pp.tile([128, HC], F32, tag="bsc")
            nc.sync.dma_start(bsc_sb[:], bsc_d.ap().rearrange("(c p) -> p c", p=128))
            bsr_sb = pp.tile([128, HC], F32, tag="bsr")
            nc.sync.dma_start(bsr_sb[:], bsr_d.ap().rearrange("(c p) -> p c", p=128))
            bsum_c = pp.tile([128, HC], F32, tag="bsum_c")
            nc.vector.tensor_tensor(bsum_c[:], bsh_sb[:], bsc_sb[:], mybir.AluOpType.add)
            bsum_r = pp.tile([128, HC], F32, tag="bsum_r")
            nc.vector.tensor_tensor(bsum_r[:], bsh_sb[:], bsr_sb[:], mybir.AluOpType.add)

            U_sb = pp.tile([128, HC, H], F32R, tag="U")
            nc.sync.dma_start(U_sb[:], U_d.ap().bitcast(F32R).rearrange("(c p) n -> p c n", p=128))

            # attention outputs, transposed: [128, hc, (j, i)]
            attT = pp.tile([128, HC, 2, BS], F32, tag="attT")
            # TP-GRU transposed input stack (f32r), full batch:
            # slots 0..3 = input (DMA), 4..11 = c_t (from AllGather)
            zin2 = pp.tile([128, KC_IN, B], F32R, tag="zin2")
            nc.sync.dma_start(zin2[:, 0:EC, :],
                              inTf.ap().bitcast(F32R).rearrange("(c p) i -> p c i", p=128))

            def zin(kc):
                return zin2[:, kc, :] if kc < KC_IN else hTf_r[:, kc - KC_IN, :]

            # =======================================================
            # Phase A0: ahT[h, i] = (W.T @ hidden_shard.T)  [H, BS]
            # =======================================================
            ahT_sb = pp.tile([128, HC * BS], F32, tag="ahT")
            with (
                tc.tile_pool(name="wstream", bufs=2) as pw,
                tc.tile_pool(name="psA0", bufs=1, space="PSUM") as psA0,
            ):
                ps_ah = psA0.tile([128, HC * BS], F32, tag="pah")
                for kc in range(HC):
                    wt = pw.tile([128, H], F32, tag="wtile")
                    nc.sync.dma_start(wt[:], W_d.ap()[kc * 128:(kc + 1) * 128, :])
                    for hc in range(HC):
                        nc.tensor.matmul(
                            ps_ah[:, hc * BS:(hc + 1) * BS],
                            lhsT=wt[:, hc * 128:(hc + 1) * 128],
                            rhs=hT_sb[:, kc, :],
                            start=(kc == 0), stop=(kc == HC - 1),
                        )
                nc.scalar.copy(ahT_sb[:], ps_ah[:])

            # =======================================================
            # Phase A: attention for j in {cnn, rnn}, batch pairs
            # =======================================================
            with (
                tc.tile_pool(name="actx", bufs=10) as pctx,
                tc.tile_pool(name="actxN", bufs=3) as pctxN,
                tc.tile_pool(name="atanh", bufs=3) as ptanh,
                tc.tile_pool(name="asmall", bufs=8) as psm,
                tc.tile_pool(name="psCU", bufs=2, space="PSUM") as psCU,
                tc.tile_pool(name="psSC", bufs=2, space="PSUM") as psSC,
                tc.tile_pool(name="psAT", bufs=1, space="PSUM") as psAT,
            ):
                for j in range(2):
                    for p in range(BS // 2):
                        i0 = 2 * p
                        # transposed context tiles for the pair, per k-chunk
                        ctx_t = []
                        for kc in range(HC):
                            t = pctx.tile([128, 2, L], F32R, tag=f"ctxT{kc % 2}")
                            nc.sync.dma_start(
                                t[:],
                                ctxT.ap().bitcast(F32R)[j, i0:i0 + 2, kc * 128:(kc + 1) * 128, :]
                                .rearrange("i p l -> p i l"),
                            )
                            ctx_t.append(t)
                        # scores psum for the pair: [1, 2*L]
                        ps_sc = psSC.tile([1, 2 * L], F32, tag="score")
                        for hc in range(HC):
                            ps_cu = psCU.tile([128, 2 * L], F32, tag="cu")
                            for kc in range(HC):
                                nc.tensor.matmul(
                                    ps_cu[:],
                                    lhsT=U_sb[:, kc, hc * 128:(hc + 1) * 128],
                                    rhs=ctx_t[kc][:],
                                    start=(kc == 0), stop=(kc == HC - 1),
                                )
                            th = ptanh.tile([128, 2 * L], BF16, tag="tanh")
                            for ii in range(2):
                                nc.scalar.activation(
                                    th[:, ii * L:(ii + 1) * L],
                                    ps_cu[:, ii * L:(ii + 1) * L],
                                    mybir.ActivationFunctionType.Tanh,
                                    bias=ahT_sb[:, hc * BS + i0 + ii:hc * BS + i0 + ii + 1],
                                )
                            nc.tensor.matmul(
                                ps_sc[:],
                                lhsT=v_bf[:, hc:hc + 1],
                                rhs=th[:],
                                start=(hc == 0), stop=(hc == HC - 1),
                            )
                        for ii in range(2):
                            i = i0 + ii
                            # softmax over L (unnormalized weights + recip sum)
                            srow = psm.tile([1, L], F32, tag="srow")
                            nc.scalar.copy(srow[:], ps_sc[:, ii * L:(ii + 1) * L])
                            nc.vector.copy_predicated(srow[:], pad_sb[0:1, i * L:(i + 1) * L],
                                                      negbig[:])
                            nmax = psm.tile([1, 1], F32, tag="nmax")
                            nc.vector.tensor_reduce(
                                nmax[:], srow[:], axis=mybir.AxisListType.X,
                                op=mybir.AluOpType.max, negate=True,
                            )
                            wrow = psm.tile([1, L], F32, tag="wrow")
                            ssum = psm.tile([1, 1], F32, tag="ssum")
                            nc.scalar.activation(
                                wrow[:], srow[:], mybir.ActivationFunctionType.Exp,
                                bias=nmax[:], accum_out=ssum[:],
                            )
                            rsum = psm.tile([1, 1], F32, tag="rsum")
                            nc.vector.reciprocal(rsum[:], ssum[:])
                            # wcol = wrow.T * (1/sum)  via PE: lhsT=wrow, rhs=rsum
                            ps_wc = psAT.tile([128, 1], F32, tag="wcol_ps")
                            nc.tensor.matmul(ps_wc[:], lhsT=wrow[:], rhs=rsum[:],
                                             start=True, stop=True)
                            wcol = psm.tile([128, 1], F32, tag="wcol")
                            nc.vector.tensor_copy(wcol[:], ps_wc[:])
                            # natural ctx for this (j, i)
                            cn = pctxN.tile([L, H], F32, tag="ctxN")
                            nc.sync.dma_start(cn[:], ctxN.ap()[j, i, :, :])
                            # attT[:, hc, j, i] = ctxN_chunk.T @ wcol
                            ps_at = psAT.tile([128, HC], F32, tag="attT_ps", bufs=2)
                            for hc in range(HC):
                                nc.tensor.matmul(
                                    ps_at[:, hc:hc + 1],
                                    lhsT=cn[:, hc * 128:(hc + 1) * 128],
                                    rhs=wcol[:],
                                    start=True, stop=True,
                                )
                            for hc in range(HC):
                                nc.vector.tensor_copy(attT[:, hc, j, i:i + 1], ps_at[:, hc:hc + 1])

            # =======================================================
            # Phase B: merge gate (T-space)
            # =======================================================
            with (
                tc.tile_pool(name="mw", bufs=3) as pmw,
                tc.tile_pool(name="msb", bufs=2) as pmsb,
                tc.tile_pool(name="psM", bufs=1, space="PSUM") as psM,
                tc.tile_pool(name="psS2", bufs=1, space="PSUM") as psS2,
            ):
                ps_sh = psM.tile([128, HC, BS], F32, tag="sh")
                ps_c = psM.tile([128, HC, BS], F32, tag="sc")
                ps_r = psM.tile([128, HC, BS], F32, tag="sr")
                for kc in range(HC):
                    wh = pmw.tile([128, H], F32, tag="wh")
                    nc.sync.dma_start(wh[:], wsh_t.ap()[kc * 128:(kc + 1) * 128, :])
                    wc = pmw.tile([128, H], F32, tag="wc")
                    nc.sync.dma_start(wc[:], wsc_t.ap()[kc * 128:(kc + 1) * 128, :])
                    wr = pmw.tile([128, H], F32, tag="wr")
                    nc.sync.dma_start(wr[:], wsr_t.ap()[kc * 128:(kc + 1) * 128, :])
                    st, sp = (kc == 0), (kc == HC - 1)
                    for hc in range(HC):
                        nc.tensor.matmul(ps_sh[:, hc, :], lhsT=wh[:, hc * 128:(hc + 1) * 128],
                                         rhs=hT_sb[:, kc, :], start=st, stop=sp)
                        nc.tensor.matmul(ps_c[:, hc, :], lhsT=wc[:, hc * 128:(hc + 1) * 128],
                                         rhs=attT[:, kc, 0, :], start=st, stop=sp)
                        nc.tensor.matmul(ps_r[:, hc, :], lhsT=wr[:, hc * 128:(hc + 1) * 128],
                                         rhs=attT[:, kc, 1, :], start=st, stop=sp)
                # tanh(c-branch), tanh(r-branch); wS dot partial scores
                sh_sb = pmsb.tile([128, HC, BS], F32, tag="sh_sb")
                nc.scalar.copy(sh_sb[:], ps_sh[:])
                tnh = pmsb.tile([128, HC, 2 * BS], F32, tag="tnh")
                for hc in range(HC):
                    for (br, ps_b, bias) in ((0, ps_c, bsum_c), (1, ps_r, bsum_r)):
                        tmp = tnh[:, hc, br * BS:(br + 1) * BS]
                        nc.vector.tensor_tensor(tmp, ps_b[:, hc, :], sh_sb[:, hc, :],
                                                mybir.AluOpType.add)
                        nc.scalar.activation(tmp, tmp, mybir.ActivationFunctionType.Tanh,
                                             bias=bias[:, hc:hc + 1])
                ps_s2 = psS2.tile([1, 2 * BS], F32, tag="s2")
                for hc in range(HC):
                    nc.tensor.matmul(ps_s2[:], lhsT=wS_sb[:, hc:hc + 1],
                                     rhs=tnh[:, hc, :], start=(hc == 0), stop=(hc == HC - 1))
                s2_sb = pmsb.tile([1, 2 * BS], F32, tag="s2_sb")
                nc.scalar.copy(s2_sb[:], ps_s2[:])
                gdiff = pmsb.tile([1, BS], F32, tag="gdiff")
                nc.vector.tensor_tensor(gdiff[:], s2_sb[:, 0:BS], s2_sb[:, BS:2 * BS],
                                        mybir.AluOpType.subtract)
                grow = pmsb.tile([1, BS], F32, tag="grow")
                nc.scalar.activation(grow[:], gdiff[:], mybir.ActivationFunctionType.Sigmoid)
                ps_gb = psS2.tile([128, BS], F32, tag="gb")
                nc.tensor.matmul(ps_gb[:], lhsT=ones[0:1, 0:128], rhs=grow[:],
                                 start=True, stop=True)
                gb = pmsb.tile([128, BS], F32, tag="gb_sb")
                nc.vector.tensor_copy(gb[:], ps_gb[:])
                # c_tT = arT + gb*(acT - arT)  [H, BS] for this core's rows
                ctT_sb = pmsb.tile([128, HC, BS], F32, tag="ctT")
                for hc in range(HC):
                    ctmp = pmsb.tile([128, BS], F32, tag="ctmp", bufs=3)
                    nc.vector.tensor_tensor(ctmp[:], attT[:, hc, 0, :], attT[:, hc, 1, :],
                                            mybir.AluOpType.subtract)
                    nc.vector.tensor_tensor(ctmp[:], ctmp[:], gb[:], mybir.AluOpType.mult)
                    nc.vector.tensor_tensor(ctT_sb[:, hc, :], ctmp[:], attT[:, hc, 1, :],
                                            mybir.AluOpType.add)
                # AllGather c_t across cores -> zin2 slots 4..11 (all 64 rows)
                ag_ct_in = pdram.tile([H, BS], F32, tag="ag_ct_in")
                ag_ct_out = pdram.tile([NC, H, BS], F32, tag="ag_ct_out")
                nc.sync.dma_start(ag_ct_in.rearrange("(c p) i -> p c i", p=128), ctT_sb[:])
                nc.gpsimd.collective_compute(
                    "AllGather", mybir.AluOpType.bypass, replica_groups=rg,
                    ins=[ag_ct_in.opt()], outs=[ag_ct_out.opt()],
                )
                for r in range(NC):
                    nc.sync.dma_start(
                        zin2[:, EC:KC_IN, r * BS:(r + 1) * BS],
                        ag_ct_out[:].bitcast(F32R)[r].rearrange("(c p) i -> p c i", p=128),
                    )

            # =======================================================
            # Phase C: GRU step, tensor-parallel over H (all 64 rows,
            # this core's 128 hidden columns)
            # =======================================================
            with (
                tc.tile_pool(name="gw", bufs=3) as pgw,
                tc.tile_pool(name="gsb", bufs=1) as pgsb,
                tc.tile_pool(name="psG", bufs=1, space="PSUM") as psG,
                tc.tile_pool(name="psTr", bufs=1, space="PSUM") as psTr,
            ):
                brz_sb = pgsb.tile([2, 256], F32R, tag="brz")
                nc.sync.dma_start(brz_sb[:], b_rz.ap().bitcast(F32R))
                bin_sb = pgsb.tile([1, 128], F32R, tag="bin")
                nc.sync.dma_start(bin_sb[:], b_in.ap().bitcast(F32R))
                bhn_sb = pgsb.tile([1, 128], F32R, tag="bhn")
                nc.sync.dma_start(bhn_sb[:], b_hn.ap().bitcast(F32R))
                hcol_sb = pgsb.tile([B, 128], F32, tag="hcol")
                nc.sync.dma_start(hcol_sb[:], hcol.ap())

                ps_rz = psG.tile([B, 256], F32, tag="ps_rz")
                for kc in range(KC_X):
                    wt = pgw.tile([128, 256], F32R, tag="gwrz")
                    nc.sync.dma_start(wt[:], w_rz.ap().bitcast(F32R)[kc * 128:(kc + 1) * 128, :])
                    nc.tensor.matmul(ps_rz[:], lhsT=zin(kc), rhs=wt[:],
                                     start=(kc == 0), stop=False)
                nc.tensor.matmul(ps_rz[:], lhsT=ones_r[0:2, 0:B], rhs=brz_sb[:],
                                 start=False, stop=True)
                rz_sb = pgsb.tile([B, 256], F32, tag="rz")
                nc.scalar.activation(rz_sb[:], ps_rz[:],
                                     mybir.ActivationFunctionType.Sigmoid)

                ps_i = psG.tile([B, 128], F32, tag="ps_i")
                for kc in range(KC_IN):
                    wt = pgw.tile([128, 128], F32R, tag="gwin")
                    nc.sync.dma_start(wt[:], w_in.ap().bitcast(F32R)[kc * 128:(kc + 1) * 128, :])
                    nc.tensor.matmul(ps_i[:], lhsT=zin(kc), rhs=wt[:],
                                     start=(kc == 0), stop=False)
                nc.tensor.matmul(ps_i[:], lhsT=ones_r[0:1, 0:B], rhs=bin_sb[:],
                                 start=False, stop=True)
                ps_h = psG.tile([B, 128], F32, tag="ps_h")
                for kc in range(HC):
                    wt = pgw.tile([128, 128], F32R, tag="gwhn")
                    nc.sync.dma_start(wt[:], w_hn.ap().bitcast(F32R)[kc * 128:(kc + 1) * 128, :])
                    nc.tensor.matmul(ps_h[:], lhsT=hTf_r[:, kc, :], rhs=wt[:],
                                     start=(kc == 0), stop=False)
                nc.tensor.matmul(ps_h[:], lhsT=ones_r[0:1, 0:B], rhs=bhn_sb[:],
                                 start=False, stop=True)
                # n = tanh(i_n + r * h_n); h_new = n + z*(h - n)   [B, 128]
                n_sb = pgsb.tile([B, 128], F32, tag="n_sb")
                nc.vector.tensor_tensor(n_sb[:], ps_h[:], rz_sb[:, 0:128],
                                        mybir.AluOpType.mult)
                nc.vector.tensor_tensor(n_sb[:], n_sb[:], ps_i[:], mybir.AluOpType.add)
                nc.scalar.activation(n_sb[:], n_sb[:], mybir.ActivationFunctionType.Tanh)
                hnew_sb = pgsb.tile([B, 128], F32, tag="hnewc")
                nc.vector.tensor_tensor(hnew_sb[:], hcol_sb[:], n_sb[:],
                                        mybir.AluOpType.subtract)
                nc.vector.tensor_tensor(hnew_sb[:], hnew_sb[:], rz_sb[:, 128:256],
                                        mybir.AluOpType.mult)
                nc.vector.tensor_tensor(hnew_sb[:], n_sb[:], hnew_sb[:],
                                        mybir.AluOpType.add)
                nc.sync.dma_start(hnew_d.ap(), hnew_sb[:])
                # transpose to [128, B] for the h_new AllGather
                ps_tr = psTr.tile([128, B], F32, tag="ps_tr")
                nc.tensor.matmul(ps_tr[:], lhsT=hnew_sb[:], rhs=idn[:],
                                 is_transpose=True, start=True, stop=True)
                hnT_sb = pgsb.tile([128, B], F32, tag="hnT")
                nc.vector.tensor_copy(hnT_sb[:], ps_tr[:])

            # =======================================================
            # Phase D: AllGather h_newT; output projection + log_softmax
            # =======================================================
            ag_in = pdram.tile([128, B], F32, tag="ag_in")
            ag_out = pdram.tile([NC, 128, B], F32, tag="ag_out")
            nc.sync.dma_start(ag_in[:], hnT_sb[:])
            nc.gpsimd.collective_compute(
                "AllGather", mybir.AluOpType.bypass, replica_groups=rg,
                ins=[ag_in.opt()], outs=[ag_out.opt()],
            )
            with (
                tc.tile_pool(name="dsb", bufs=1) as pdsb,
                tc.tile_pool(name="wout", bufs=6) as pwo,
                tc.tile_pool(name="dscr", bufs=3) as pscr,
                tc.tile_pool(name="psL", bufs=3, space="PSUM") as psL,
            ):
                # hT64[:, kc, :] = core kc's h_newT shard (already transposed)
                hT64 = pdsb.tile([128, HC, B], F32R, tag="hT64")
                nc.sync.dma_start(hT64[:],
                                  ag_out[:].bitcast(F32R).rearrange("r p b -> p r b"))

                bout_sb = pdsb.tile([1, VS], F32R, tag="bout")
                nc.sync.dma_start(bout_sb[:], bout.ap().bitcast(F32R))
                logits = pdsb.tile([B, VS], F32, tag="logits")
                vb = _vblocks()
                nbv = len(vb)
                mx = pdsb.tile([B, nbv], F32, tag="mx")
                sx = pdsb.tile([B, nbv], F32, tag="sx")
                for bi, (o, n) in enumerate(vb):
                    ps_l = psL.tile([B, 512], F32, tag="lps")
                    for kc in range(HC):
                        wt = pwo.tile([128, 512], F32R, tag="wot")
                        nc.sync.dma_start(wt[:, 0:n],
                                          wout.ap().bitcast(F32R)[kc * 128:(kc + 1) * 128, o:o + n])
                        nc.tensor.matmul(ps_l[:, 0:n], lhsT=hT64[:, kc, :],
                                         rhs=wt[:, 0:n], start=(kc == 0), stop=False)
                    nc.tensor.matmul(ps_l[:, 0:n], lhsT=ones_r[0:1, 0:B],
                                     rhs=bout_sb[:, o:o + n], start=False, stop=True)
                    nc.scalar.copy(logits[:, o:o + n], ps_l[:, 0:n])
                    nc.vector.tensor_reduce(mx[:, bi:bi + 1], ps_l[:, 0:n],
                                            axis=mybir.AxisListType.X, op=mybir.AluOpType.max)
                nmx = pdsb.tile([B, 1], F32, tag="nmx")
                nc.vector.tensor_reduce(nmx[:], mx[:], axis=mybir.AxisListType.X,
                                        op=mybir.AluOpType.max, negate=True)
                pmx = pdsb.tile([B, 1], F32, tag="pmx")
                nc.scalar.mul(pmx[:], nmx[:], -1.0)
                for bi, (o, n) in enumerate(vb):
                    scr = pscr.tile([B, 512], F32, tag="scr")
                    nc.scalar.activation(scr[:, 0:n], logits[:, o:o + n],
                                         mybir.ActivationFunctionType.Exp,
                                         bias=nmx[:], accum_out=sx[:, bi:bi + 1])
                sloc = pdsb.tile([B, 1], F32, tag="sloc")
                nc.vector.tensor_reduce(sloc[:], sx[:], axis=mybir.AxisListType.X,
                                        op=mybir.AluOpType.add)
                # pack partials [B, 2] = (max, sumexp); AllGather; combine
                part = pdsb.tile([B, 2], F32, tag="part")
                nc.vector.tensor_copy(part[:, 0:1], pmx[:])
                nc.vector.tensor_copy(part[:, 1:2], sloc[:])
                ag2_in = pdram.tile([B, 2], F32, tag="ag2_in")
                ag2_out = pdram.tile([NC, B, 2], F32, tag="ag2_out")
                nc.sync.dma_start(ag2_in[:], part[:])
                nc.gpsimd.collective_compute(
                    "AllGather", mybir.AluOpType.bypass, replica_groups=rg,
                    ins=[ag2_in.opt()], outs=[ag2_out.opt()],
                )
                gath = pdsb.tile([B, 2, NC], F32, tag="gath")
                nc.sync.dma_start(gath[:], ag2_out[:].rearrange("r b c -> b c r"))
                gnm = pdsb.tile([B, 1], F32, tag="gnm")
                nc.vector.tensor_reduce(gnm[:], gath[:, 0:1, :], axis=mybir.AxisListType.X,
                                        op=mybir.AluOpType.max, negate=True)
                gpm = pdsb.tile([B, 1], F32, tag="gpm")
                nc.scalar.mul(gpm[:], gnm[:], -1.0)
                # sum_r s_r * exp(m_r - M)
                et = pdsb.tile([B, NC], F32, tag="et")
                nc.scalar.activation(et[:], gath[:, 0, :], mybir.ActivationFunctionType.Exp,
                                     bias=gnm[:])
                nc.vector.tensor_tensor(et[:], et[:], gath[:, 1, :], mybir.AluOpType.mult)
                gs = pdsb.tile([B, 1], F32, tag="gs")
                nc.vector.tensor_reduce(gs[:], et[:], axis=mybir.AxisListType.X,
                                        op=mybir.AluOpType.add)
                lng = pdsb.tile([B, 1], F32, tag="lng")
                nc.scalar.activation(lng[:], gs[:], mybir.ActivationFunctionType.Ln)
                nlz = pdsb.tile([B, 1], F32, tag="nlz")
                nc.vector.tensor_tensor(nlz[:], gpm[:], lng[:], mybir.AluOpType.add)
                nc.scalar.mul(nlz[:], nlz[:], -1.0)
                # logp = logits - logZ ; single pass then DMA out
                nc.scalar.activation(logits[:], logits[:],
                                     mybir.ActivationFunctionType.Identity, bias=nlz[:])
                nc.sync.dma_start(logp_d.ap(), logits[:])

    nc.finalize()
    return nc


_NC_CACHE = None


def _get_nc():
    global _NC_CACHE
    if _NC_CACHE is None:
        _NC_CACHE = build_kernel()
    return _NC_CACHE


def make_in_maps(inputs):
    """Shard + lay out the full inputs into per-core input maps."""
    f = np.ascontiguousarray
    inp = np.asarray(inputs["input"], np.float32)          # [B,1,E]
    hid = np.asarray(inputs["hidden"], np.float32)         # [1,B,H]
    cc = np.asarray(inputs["context_hiddens_cnn"], np.float32)
    cr = np.asarray(inputs["context_hiddens_rnn"], np.float32)
    pad = np.asarray(inputs["pad_matrix"]).astype(np.uint8)
    W = np.asarray(inputs["W"], np.float32)
    U = np.asarray(inputs["U"], np.float32)
    v = np.asarray(inputs["v"], np.float32)
    WSh_w = np.asarray(inputs["WSh_w"], np.float32)
    WSh_b = np.asarray(inputs["WSh_b"], np.float32)
    WSc_w = np.asarray(inputs["WSc_w"], np.float32)
    WSc_b = np.asarray(inputs["WSc_b"], np.float32)
    WSr_w = np.asarray(inputs["WSr_w"], np.float32)
    WSr_b = np.asarray(inputs["WSr_b"], np.float32)
    wS_w = np.asarray(inputs["wS_w"], np.float32)
    W_ih = np.asarray(inputs["W_ih"], np.float32)
    W_hh = np.asarray(inputs["W_hh"], np.float32)
    b_ih = np.asarray(inputs["b_ih"], np.float32)
    b_hh = np.asarray(inputs["b_hh"], np.float32)
    W_out = np.asarray(inputs["W_out"], np.float32)
    b_out = np.asarray(inputs["b_out"], np.float32)

    # shared (replicated) weight layouts
    wsh_t = f(WSh_w.T)
    wsc_t = f(WSc_w.T)
    wsr_t = f(WSr_w.T)
    wS_col = f(wS_w[0][:, None])
    w_rz_t = np.concatenate([W_ih[:G2, :], W_hh[:G2, :]], axis=1).T  # [2560, 2048]
    w_in_t = W_ih[G2:, :].T    # [1536, 1024]
    w_hn_t = W_hh[G2:, :].T    # [1024, 1024]
    b_rz2 = np.stack([b_ih[:G2], b_hh[:G2]], axis=0)
    b_in1 = b_ih[None, G2:]
    b_hn1 = b_hh[None, G2:]
    hTf = f(hid[0].T)          # [H, B]
    inTf = f(inp[:, 0, :].T)   # [E, B]

    # padded vocab shards
    Wout_p = np.zeros((VP, H), np.float32)
    Wout_p[:V] = W_out
    bout_p = np.full((VP,), NEG_BIG, np.float32)
    bout_p[:V] = b_out

    ctx2 = np.stack([cc, cr], axis=0)  # [2, B, L, H]

    maps = []
    for k in range(NC):
        bs = slice(k * BS, (k + 1) * BS)
        vs = slice(k * VS, (k + 1) * VS)
        hs = slice(k * 128, (k + 1) * 128)
        m = {
            "ctxT": f(ctx2[:, bs].transpose(0, 1, 3, 2)),
            "ctxN": f(ctx2[:, bs]),
            "hT": f(hid[0, bs].T),
            "hTf": hTf,
            "inTf": inTf,
            "hcol": f(hid[0, :, hs]),
            "pad": f(pad[bs].reshape(1, -1)),
            "U": U, "W": W, "v": v,
            "wsh_t": wsh_t, "wsc_t": wsc_t, "wsr_t": wsr_t,
            "bsh": WSh_b, "bsc": WSc_b, "bsr": WSr_b,
            "wS": wS_col,
            "w_rz_k": f(np.concatenate([w_rz_t[:, hs], w_rz_t[:, H + k * 128:H + (k + 1) * 128]],
                                       axis=1)),
            "w_in_k": f(w_in_t[:, hs]),
            "w_hn_k": f(w_hn_t[:, hs]),
            "b_rz_k": f(np.concatenate([b_rz2[:, hs], b_rz2[:, H + k * 128:H + (k + 1) * 128]],
                                       axis=1)),
            "b_in_k": f(b_in1[:, hs]),
            "b_hn_k": f(b_hn1[:, hs]),
            "w_out_t": f(Wout_p[vs].T),
            "b_out": f(bout_p[None, vs]),
        }
        maps.append(m)
    return maps


def assemble(results):
    logp = np.empty((B, VP), np.float32)
    hnew = np.empty((B, H), np.float32)
    for k in range(NC):
        logp[:, k * VS:(k + 1) * VS] = results[k]["logp"]
        hnew[:, k * 128:(k + 1) * 128] = results[k]["hnewc"]
    return logp[:, :V], hnew[None]


def kernel(**inputs):
    nc = _get_nc()
    in_maps = make_in_maps(inputs)
    res = run_bass_kernel_spmd(nc, in_maps, core_ids=list(range(NC)))
    return assemble(res.results)
